# revision 17
# baseline (speedup 1.0000x reference)
"""Additive attention (Bahdanau) Trainium2 Bass kernel.

Math (per batch b):
    q' = queries @ W_q            (Q, H)   -> stored transposed [H, Q]
    k' = keys @ W_k               (K, H)   -> stored transposed [H, K]
    scores[q,k] = sum_h w_v[h] * tanh(q'[h,q] + k'[h,k])   (+ -1e9 mask tail)
    out = softmax(scores) @ values

Distribution: work item = (batch, 128-wide key chunk, 128-wide query block);
only key chunks intersecting [0, valid_len_b) exist.  Items are dealt
round-robin onto 8 cores (identical instruction stream - pure SPMD; cores
differ only through input data).  Each item emits unnormalized flash partials
PV = exp(S) @ V and l = rowsum(exp(S)); the host sums partials over key
chunks and normalizes.  No running max is needed: |scores| <= sum|w_v| ~ 9,
so exp() cannot overflow fp32.

Device pipeline per item:
    PE    : q'/k' projections (bf16 operands, fp32 PSUM accumulation)
    DVE   : S_pre[h, q, k] = q'[h,q] + k'[h,k]  (bf16 broadcast add).
            q' is stored pair-duplicated [H, NQ, 2] so every operand AP has
            an innermost unit-stride pair -> DVE picks the 2x_1P bf16 mode.
    ACT   : T = tanh(S_pre) -> bf16              (one big activation)
    PE    : per query q: load T[:,q,:] as the STATIONARY operand and stream
            w_v (N=1) -> writes the scores column ps_st[:, q].  This yields
            the scores tile already TRANSPOSED [k, q] in fp32 PSUM with no
            per-query weight-window matmuls and no later transpose.
    ACT   : p[k, q] = exp(ps_st + mask[k])  (the -1e9 tail mask is the
            per-partition activation bias) -> bf16
    PE    : load p as stationary; stream V-chunk -> PV [q, VD], and stream
            a ones-column -> l [q, 1] (the exp row sums).

bf16 is used where an operand merely streams through the PE array (fp32
moving operands stream at 1/4 rate) or where DVE's 2x bf16 mode doubles
elementwise throughput; all reductions/accumulations stay fp32 (PSUM).
"""

import functools
import math

import numpy as np

import concourse.bacc as bacc
import concourse.bass as bass
import concourse.tile as tile
from concourse import mybir
from concourse.bass_utils import run_bass_kernel_spmd

N_CORES = 8
B, Q, K, D, VD, H = 4, 512, 1024, 256, 256, 128
KC = 128          # keys per item
NQ = 128          # queries per item
NQB = Q // NQ     # q-blocks per (batch, kchunk)
NEG = -1e9

F32 = mybir.dt.float32
BF16 = mybir.dt.bfloat16
NP_BF16 = mybir.dt.np(BF16)

# Results of the last device run (for the test harness to inspect timing).
LAST_RESULTS = None


def _ensure_axon_hooks():
    """run_bass_kernel_spmd(trace=True) imports antenv.axon_hooks, which not
    every container image ships.  Provide a no-op fallback so a BASS_TRACE=1
    environment degrades to an untraced run instead of crashing."""
    try:
        import antenv.axon_hooks  # noqa: F401
    except ImportError:
        import sys
        import types

        mod = types.ModuleType("antenv.axon_hooks")
        mod.get_axon_ntff_profile_hook = lambda: None
        mod.set_axon_ntff_profile_hook = lambda h: None
        sys.modules["antenv.axon_hooks"] = mod


@functools.lru_cache(maxsize=None)
def _build_program(ni: int):
    """Build the Bass program for `ni` work items per core."""
    nc = bacc.Bacc("TRN2", target_bir_lowering=False, debug=False, num_devices=N_CORES)

    kT = nc.declare_dram_parameter("kT", [ni, D, KC], BF16, isOutput=False)
    qT = nc.declare_dram_parameter("qT", [ni, D, NQ], BF16, isOutput=False)
    vv = nc.declare_dram_parameter("vv", [ni, KC, VD], BF16, isOutput=False)
    msk = nc.declare_dram_parameter("msk", [ni, KC, 1], F32, isOutput=False)
    wq = nc.declare_dram_parameter("wq", [D, H], BF16, isOutput=False)
    wk = nc.declare_dram_parameter("wk", [D, H], BF16, isOutput=False)
    wvc = nc.declare_dram_parameter("wvc", [H, 1], BF16, isOutput=False)
    onesc = nc.declare_dram_parameter("onesc", [KC, 1], BF16, isOutput=False)

    pv = nc.declare_dram_parameter("pv", [ni, NQ, VD], F32, isOutput=True)
    ls = nc.declare_dram_parameter("ls", [ni, NQ, 1], F32, isOutput=True)

    DT = D // 128  # d-dim tiles (2)
    add = mybir.AluOpType.add
    Tanh = mybir.ActivationFunctionType.Tanh
    Exp = mybir.ActivationFunctionType.Exp

    with tile.TileContext(nc) as tc:
        with (
            tc.tile_pool(name="consts", bufs=1) as consts,
            tc.tile_pool(name="item", bufs=3) as item,
            tc.tile_pool(name="proj", bufs=2) as proj,
            tc.tile_pool(name="spre", bufs=2) as spre_pool,
            tc.tile_pool(name="tnh", bufs=2) as tnh_pool,
            tc.tile_pool(name="small", bufs=4) as small,
            tc.tile_pool(name="psq", bufs=1, space="PSUM") as psq_pool,
            tc.tile_pool(name="psk", bufs=2, space="PSUM") as psk_pool,
            tc.tile_pool(name="pss", bufs=2, space="PSUM") as pss_pool,
            tc.tile_pool(name="psl", bufs=1, space="PSUM") as psl_pool,
            tc.tile_pool(name="pso", bufs=2, space="PSUM") as pso_pool,
        ):
            sb_wq = consts.tile([128, DT, H], BF16)
            sb_wk = consts.tile([128, DT, H], BF16)
            sb_wvc = consts.tile([H, 1], BF16)
            sb_onesc = consts.tile([KC, 1], BF16)

            def load_consts():
                # wq/wk gate the projections -> issue before the rest; the
                # Sync HWDGE ring transfers one DMA at a time.
                nc.sync.dma_start(
                    out=sb_wk, in_=wk[:].rearrange("(t p) h -> p t h", p=128)
                )
                nc.sync.dma_start(
                    out=sb_wq, in_=wq[:].rearrange("(t p) h -> p t h", p=128)
                )
                nc.sync.dma_start(out=sb_wvc, in_=wvc[:])
                nc.sync.dma_start(out=sb_onesc, in_=onesc[:])

            for it in range(ni):
                sb_kT = item.tile([128, DT, KC], BF16, tag="kT")
                nc.sync.dma_start(
                    out=sb_kT, in_=kT[it].rearrange("(t p) k -> p t k", p=128)
                )
                sb_qT = item.tile([128, DT, NQ], BF16, tag="qT")
                nc.sync.dma_start(
                    out=sb_qT, in_=qT[it].rearrange("(t p) q -> p t q", p=128)
                )
                if it == 0:
                    load_consts()
                sb_v = item.tile([KC, VD], BF16, tag="v")
                nc.sync.dma_start(out=sb_v, in_=vv[it])
                sb_msk = item.tile([KC, 1], F32, tag="msk")
                nc.sync.dma_start(out=sb_msk, in_=msk[it])

                # projections: q'^T [H, NQ] (pair-duplicated), k'^T [H, KC]
                ps_q = psq_pool.tile([H, NQ], F32)
                for t in range(DT):
                    nc.tensor.matmul(
                        ps_q, lhsT=sb_wq[:, t, :], rhs=sb_qT[:, t, :],
                        start=(t == 0), stop=(t == DT - 1),
                    )
                # qp2[h, q, j] = q'[h, q] for j in {0, 1}: the duplicated pair
                # gives the broadcast-add a unit-stride innermost dimension.
                qp2 = proj.tile([H, NQ, 2], BF16, tag="qp")
                nc.vector.tensor_copy(
                    qp2, ps_q[:].unsqueeze(2).broadcast_to((H, NQ, 2))
                )

                ps_k = psk_pool.tile([H, KC], F32)
                for t in range(DT):
                    nc.tensor.matmul(
                        ps_k, lhsT=sb_wk[:, t, :], rhs=sb_kT[:, t, :],
                        start=(t == 0), stop=(t == DT - 1),
                    )
                sb_kp = proj.tile([H, KC], BF16, tag="kp")
                nc.vector.tensor_copy(sb_kp, ps_k)

                def process_block(q0: int, nb: int):
                    """Full pipeline (add->tanh->scores->exp->PV) for queries
                    [q0, q0+nb) of the current item."""
                    qs = slice(q0, q0 + nb)
                    # S_pre[h, q, (a,b)] = q'[h, q] + k'[h, 2a+b]  (bf16, 2x)
                    spre = spre_pool.tile([H, nb, KC], BF16, tag="spre")
                    nc.vector.tensor_tensor(
                        spre[:].rearrange("h q (a b) -> h q a b", b=2),
                        sb_kp[:]
                        .rearrange("h (a b) -> h a b", b=2)
                        .unsqueeze(1)
                        .broadcast_to((H, nb, KC // 2, 2)),
                        qp2[:, qs].unsqueeze(2).broadcast_to((H, nb, KC // 2, 2)),
                        op=add,
                    )
                    tnh = tnh_pool.tile([H, nb, KC], BF16, tag="tnh")
                    nc.scalar.activation(tnh, spre, Tanh)

                    # scores^T[k, q] = sum_h T[h, q, k] * w_v[h]: T[:, q, :]
                    # is the stationary operand, w_v the (N=1) moving one.
                    ps_st = pss_pool.tile([KC, nb], F32, tag="pss")
                    for q in range(nb):
                        nc.tensor.matmul(
                            ps_st[:, q: q + 1], lhsT=tnh[:, q, :], rhs=sb_wvc,
                            start=True, stop=True,
                        )

                    # p = exp(scores^T + mask[k]): the key-tail mask is a
                    # per-partition activation bias in this layout.
                    p_t = small.tile([KC, nb], BF16, tag="p")
                    nc.scalar.activation(p_t, ps_st, Exp, bias=sb_msk)

                    # One weight load of p serves both matmuls:
                    #   PV[q, v] = sum_k p[k, q] * V[k, v]
                    #   l[q]     = sum_k p[k, q]
                    ps_o = pso_pool.tile([nb, VD], F32, tag="pso")
                    nc.tensor.matmul(ps_o, lhsT=p_t, rhs=sb_v, start=True, stop=True)
                    ps_l = psl_pool.tile([nb, 1], F32, tag="psl")
                    nc.tensor.matmul(ps_l, lhsT=p_t, rhs=sb_onesc, start=True, stop=True)

                    sb_o = small.tile([nb, VD], F32, tag="o")
                    nc.vector.tensor_copy(sb_o, ps_o)
                    sb_l = small.tile([nb, 1], F32, tag="l")
                    nc.vector.tensor_copy(sb_l, ps_l)

                    nc.sync.dma_start(out=pv[it, qs], in_=sb_o)
                    nc.sync.dma_start(out=ls[it, qs], in_=sb_l)

                # Smaller leading blocks shorten the pipeline ramp on the
                # first item; smaller trailing blocks shorten the drain on
                # the last one.
                if it == 0 and ni == 1:
                    blocks = [32, 32, 32, 16, 16]
                elif it == 0:
                    blocks = [16, 16, 32, 64]
                elif it == ni - 1:
                    blocks = [64, 32, 16, 16]
                else:
                    blocks = [NQ]
                q0 = 0
                for nb in blocks:
                    process_block(q0, nb)
                    q0 += nb

    if not nc.is_finalized():
        nc.finalize()
    return nc


def kernel(queries, keys, values, valid_lens, W_q, W_k, w_v):
    global LAST_RESULTS
    queries = np.ascontiguousarray(np.asarray(queries, dtype=np.float32))
    keys = np.ascontiguousarray(np.asarray(keys, dtype=np.float32))
    values = np.ascontiguousarray(np.asarray(values, dtype=np.float32))
    vl = np.asarray(valid_lens).astype(np.int64)
    W_q = np.asarray(W_q, dtype=np.float32)
    W_k = np.asarray(W_k, dtype=np.float32)
    w_v = np.asarray(w_v, dtype=np.float32)

    # ---- plan work items -------------------------------------------------
    items = []  # (b, kc, qb)
    for b in range(B):
        for kc in range(int(math.ceil(vl[b] / KC))):
            for qb in range(NQB):
                items.append((b, kc, qb))
    n_real = len(items)
    ni = (n_real + N_CORES - 1) // N_CORES
    while len(items) < ni * N_CORES:
        items.append(items[0])  # dummy duplicate, ignored at merge time

    core_items = [[items[c + N_CORES * j] for j in range(ni)] for c in range(N_CORES)]

    # ---- shared constant tensors ----------------------------------------
    wvc = w_v.reshape(H, 1).astype(NP_BF16)
    onesc = np.ones((KC, 1), dtype=NP_BF16)

    qTb = [np.ascontiguousarray(queries[b].T).astype(NP_BF16) for b in range(B)]
    kTb = [np.ascontiguousarray(keys[b].T).astype(NP_BF16) for b in range(B)]
    v_bf = values.astype(NP_BF16)

    in_maps = []
    for c in range(N_CORES):
        kT = np.empty((ni, D, KC), dtype=NP_BF16)
        qT = np.empty((ni, D, NQ), dtype=NP_BF16)
        vv = np.empty((ni, KC, VD), dtype=NP_BF16)
        msk = np.empty((ni, KC, 1), dtype=np.float32)
        for j, (b, kc, qb) in enumerate(core_items[c]):
            sl = slice(kc * KC, (kc + 1) * KC)
            kT[j] = kTb[b][:, sl]
            qT[j] = qTb[b][:, qb * NQ:(qb + 1) * NQ]
            vv[j] = v_bf[b, sl, :]
            msk[j, :, 0] = np.where(
                np.arange(kc * KC, (kc + 1) * KC) < vl[b], 0.0, NEG
            ).astype(np.float32)
        in_maps.append(
            {
                "kT": kT, "qT": qT, "vv": vv, "msk": msk,
                "wq": W_q.astype(NP_BF16), "wk": W_k.astype(NP_BF16),
                "wvc": wvc, "onesc": onesc,
            }
        )

    # ---- run on the 8 cores ---------------------------------------------
    _ensure_axon_hooks()
    nc = _build_program(ni)

    def run_and_merge():
        global LAST_RESULTS
        res = run_bass_kernel_spmd(nc, in_maps, list(range(N_CORES)))
        LAST_RESULTS = res
        num = np.zeros((B, Q, VD), dtype=np.float64)
        den = np.zeros((B, Q), dtype=np.float64)
        for c in range(N_CORES):
            pv = np.asarray(res.results[c]["pv"])  # [ni, NQ, VD]
            lsum = np.asarray(res.results[c]["ls"])  # [ni, NQ, 1]
            for j, (b, kc, qb) in enumerate(core_items[c]):
                if c + N_CORES * j >= n_real:
                    continue  # dummy padding item
                blk = slice(qb * NQ, (qb + 1) * NQ)
                num[b, blk] += pv[j]
                den[b, blk] += lsum[j].reshape(NQ)
        return num, den

    num, den = run_and_merge()
    # A row sum of exp(scores) is >= exp(-|w_v|_1) > 1e-6 whenever at least
    # one key is valid (valid_lens >= 1), and everything must be finite.
    # A violation means a transient device fault - retry once.
    if not (np.isfinite(num).all() and np.isfinite(den).all() and (den > 1e-30).all()):
        num, den = run_and_merge()
    return (num / den[:, :, None]).astype(np.float32)
